# revision 1
# baseline (speedup 1.0000x reference)
"""AdaMoLE forward on 8 Trainium2 NeuronCores (Bass/Tile), data-parallel over tokens.

Reference computation (per token n):
  logits = x @ router_w.T + router_b            [N, E]
  gate   = softmax(logits)                      [N, E]
  thr    = sigmoid(x @ thr_w.T + thr_b)/E       [N, 1]
  w      = relu(gate - thr); w /= max(sum(w), eps-guard)
  h      = einsum('nd,erd->ner', x, lora_A)
  out    = einsum('ner,eor->no', h * w[:, :, None], lora_B) * SCALING

Device mapping (per core, NL=1024 tokens, two token halves pipelined):
  x streams token-half-major (half A's full d-range first), batched 4 d-chunks
  per DMA so the SP sequencer stays light. Per half:
    phase1: hT[er=128, tok] and lgT[33, tok] accumulate in PSUM over 32
            d-chunks (router cols 0:8, threshold col at partition 32 so the
            Sigmoid read is 32-aligned; biases enter via the ACT bias port)
    gating: in the [e, tok] domain with no transposes —
            w ∝ relu(exp(logits) - sum_e(exp)·thr·MAX_T) (the softmax
            denominator cancels in the normalization); partition sums and
            broadcasts are tiny ones/δ-matrix matmuls; the /16 from summing
            all 128 expanded rows is folded into B2's host-side scale
    phase2: whT = hT · wnT;  out[tok, o] = whT.T @ B2, PSUM->SBUF copies split
            DVE/ACT, output shipped bf16 and upcast on the host
  Half A's gating/phase2/out-DMA overlap half B's phase-1 matmuls.

All big operands are host-pre-transposed (and cast to bf16) so every DMA is
contiguous and the PE runs at full bf16 rate. PSUM accumulation is f32.
Phase-2 writes PSUM in [128, 1024] tiles (one copy per two matmuls keeps the
ACT/DVE instruction count low); gating PSUM tiles reuse freed accumulator
slots so they never contend with phase-2's po rotation.
Out-DMAs fire in 4 x 0.25 MB slices per token chunk so the write stream
starts as soon as the first copy lands.
Measured 76.6 us/core steady-state median, best samples ~67 us (~164 us for
the naive d-major first cut); cost-model (TimelineSim) estimate 74.8 us with
scheduler-balanced (nc.any) phase-2 copies; HBM floor ~51 us + ~6 us tail. The remaining gap is the serial gating-chain
latency (~1.4 us per engine-to-engine semaphore hop x 8 algebraically-required
ops). A reciprocal-fold into the phase-2 copy scale via DMA-transpose was
tried and REVERTED: the xbar transposes serialize against SBUF-SBUF traffic
and cost more than the two hops they save (model: 87 vs 77 us).
"""

import numpy as np

N_TOKENS = 8192
D_IN = 4096
D_OUT = 4096
E = 8
R = 16
ER = E * R  # 128
SCALING = 32.0 / 16.0
MAX_THRESHOLD = 1.0 / E
N_CORES = 8
NL = N_TOKENS // N_CORES  # tokens per core
DC = D_IN // 128  # 32 d-chunks
TCH = NL // 128  # 8 token chunks per core
OC = D_OUT // 512  # 8 output column chunks

_CACHE = {}

# tuning knobs (read at _build_nc time)
COPY_ACT_EVERY = 2  # 1 of every N phase-2 PSUM->SBUF copies goes to ScalarE
A2_DMA_SPLIT = 4  # weight DMA split count (lets phase-1 start earlier)
OUT_BF16 = True  # ship the output as bf16 and upcast on the host
B_SPLIT = 16  # d-chunks of half-B phase1 issued before gating-A (PE order)
XT_GRP = 4  # d-chunks per xt DMA
OSB_BUFS = 4
OUT_SPLIT = 4
COPY_MODE = "any"
WPOOL_BUFS = 2


def _build_nc(dbg=False, repeat=1, ablate=frozenset()):
    import concourse.mybir as mybir
    import concourse.tile as tile
    from concourse import bacc

    f32 = mybir.dt.float32
    bf16 = mybir.dt.bfloat16
    AF = mybir.ActivationFunctionType
    ALU = mybir.AluOpType

    nc = bacc.Bacc("TRN2", target_bir_lowering=False, debug=False)
    dbg_t = {}
    if dbg:
        dbg_t["lg"] = nc.declare_dram_parameter("dbg_lg", [9, NL], f32, isOutput=True)
        dbg_t["h"] = nc.declare_dram_parameter("dbg_h", [128, NL], f32, isOutput=True)
        dbg_t["wn"] = nc.declare_dram_parameter("dbg_wn", [E, NL], f32, isOutput=True)
        dbg_t["wh"] = nc.declare_dram_parameter("dbg_wh", [128, NL], f32, isOutput=True)

    xT = nc.declare_dram_parameter("xT", [DC, 128, NL], bf16, isOutput=False)
    A2 = nc.declare_dram_parameter("A2", [DC, 128, ER], bf16, isOutput=False)
    W9 = nc.declare_dram_parameter("W9", [DC, 128, 40], bf16, isOutput=False)
    B9 = nc.declare_dram_parameter("B9", [33, 1], f32, isOutput=False)
    B2 = nc.declare_dram_parameter("B2", [ER, D_OUT], bf16, isOutput=False)
    # G[0:8, (e,r)] = delta_{e,e'}; G[8, :] = -1  (expand-and-subtract matrix)
    G9 = nc.declare_dram_parameter("G9", [9, ER], bf16, isOutput=False)
    out_dt = bf16 if OUT_BF16 else f32
    out = nc.declare_dram_parameter("out", [NL, D_OUT], out_dt, isOutput=True)

    NH = NL // 2  # tokens per half
    HG = NH // 512  # 512-col groups per half (1)

    with tile.TileContext(nc) as tc:
        with (
            tc.tile_pool(name="const", bufs=1) as cpool,
            tc.tile_pool(name="xt", bufs=2 * DC // XT_GRP) as xpool,
            tc.tile_pool(name="work", bufs=WPOOL_BUFS) as wpool,
            tc.tile_pool(name="osb", bufs=OSB_BUFS) as opool,
            tc.tile_pool(name="accps", bufs=4, space="PSUM") as acc_ps,
            tc.tile_pool(name="gatps", bufs=2, space="PSUM") as gat_ps,
        ):
          out_ps = gat_ps
          for _rep in range(repeat):
            # ---- weights into SBUF (xt stream gets queue priority via order:
            # a2/w9 first, b2/g9 later so phase-1 starts early) ----
            a2_sb = cpool.tile([128, DC, ER], bf16)
            nc.sync.dma_start(
                out=a2_sb[:, 0 : DC // 4, :],
                in_=A2.ap()[0 : DC // 4].rearrange("a b c -> b a c"),
            )
            w9_sb = cpool.tile([128, DC, 40], bf16)
            nc.sync.dma_start(out=w9_sb[:], in_=W9.ap().rearrange("a b c -> b a c"))
            b9_sb = cpool.tile([33, 1], f32)
            nc.sync.dma_start(out=b9_sb[:], in_=B9.ap())
            ones_sb = cpool.tile([128, 512], bf16)
            nc.vector.memset(ones_sb[:], 1.0)
            g9_sb = cpool.tile([9, ER], bf16)
            b2_sb = cpool.tile([ER, D_OUT], bf16)

            # ---- phase 1: stream x token-half-major; half A's full d-range
            # arrives first so gating/phase2/out of half A overlap half B ----
            h = {}
            lg = {}
            for half in ("A", "B"):
                h[half] = acc_ps.tile([128, NH], f32, tag="acc", name=f"h_{half}")
                lg[half] = acc_ps.tile([128, NH], f32, tag="acc", name=f"lg_{half}")
            xts = {}
            GRP = XT_GRP  # d-chunks per xt DMA (batching keeps the SP seq light)
            for hi, half in enumerate(("A", "B")):
                tsl = slice(hi * NH, (hi + 1) * NH)
                for dg in range(DC // GRP):
                    xt = xpool.tile(
                        [128, GRP, NH], bf16, tag="xt", name=f"xt{half}{dg}"
                    )
                    nc.sync.dma_start(
                        out=xt[:],
                        in_=xT.ap()[dg * GRP : (dg + 1) * GRP, :, tsl].rearrange(
                            "a b c -> b a c"
                        ),
                    )
                    for j in range(GRP):
                        xts[half, dg * GRP + j] = xt[:, j, :]
                    if half == "A":
                        for j in range(GRP):
                            dc = dg * GRP + j
                            nc.tensor.matmul(
                                h["A"][:, :], a2_sb[:, dc, :], xts["A", dc],
                                start=(dc == 0), stop=(dc == DC - 1),
                            )
                            nc.tensor.matmul(
                                lg["A"][0:33, :], w9_sb[:, dc, 0:33], xts["A", dc],
                                start=(dc == 0), stop=(dc == DC - 1),
                            )
                        if dg == 0:
                            nc.sync.dma_start(
                                out=a2_sb[:, DC // 4 :, :],
                                in_=A2.ap()[DC // 4 :].rearrange("a b c -> b a c"),
                            )

            # ---- gating + phase 2 per half ----
            nc.sync.dma_start(out=g9_sb[:], in_=G9.ap())
            nc.sync.dma_start(out=b2_sb[:], in_=B2.ap())

            def _phase1_b(rng):
                for dc in rng:
                    nc.tensor.matmul(
                        h["B"][:, :], a2_sb[:, dc, :], xts["B", dc],
                        start=(dc == 0), stop=(dc == DC - 1),
                    )
                    nc.tensor.matmul(
                        lg["B"][0:33, :], w9_sb[:, dc, 0:33], xts["B", dc],
                        start=(dc == 0), stop=(dc == DC - 1),
                    )

            _phase1_b(range(0, B_SPLIT))

            for hi, half in enumerate(("A", "B")):
                if half == "B":
                    _phase1_b(range(B_SPLIT, DC))
                h_acc, lg_acc = h[half], lg[half]
                ex_sb = wpool.tile([E, NH], bf16, tag="ex")
                thr_sb = wpool.tile([1, NH], f32, tag="thr")
                nc.scalar.activation(
                    ex_sb[:], lg_acc[0:8, :], AF.Exp, bias=b9_sb[0:8, 0:1]
                )
                nc.scalar.activation(
                    thr_sb[:], lg_acc[32:33, :], AF.Sigmoid, bias=b9_sb[32:33, 0:1]
                )
                s_ps = acc_ps.tile([128, NH], f32, tag="acc")
                nc.tensor.matmul(
                    s_ps[0:1, :], ones_sb[0:8, 0:1], ex_sb[:, :],
                    start=True, stop=True,
                )
                nsthr_sb = wpool.tile([1, NH], bf16, tag="nsthr")
                nc.vector.scalar_tensor_tensor(
                    out=nsthr_sb[:], in0=s_ps[0:1, :], scalar=-MAX_THRESHOLD,
                    in1=thr_sb[:], op0=ALU.mult, op1=ALU.mult,
                )
                p1_ps = acc_ps.tile([128, NH], f32, tag="acc")
                nc.tensor.matmul(
                    p1_ps[:, :], g9_sb[0:8, :], ex_sb[:, :],
                    start=True, stop=False,
                )
                nc.tensor.matmul(
                    p1_ps[:, :], ones_sb[0:1, 0:128], nsthr_sb[:, :],
                    start=False, stop=True,
                )
                wtexp_sb = wpool.tile([128, NH], bf16, tag="wtexp")
                nc.vector.tensor_scalar_max(wtexp_sb[:], p1_ps[:], 0.0)
                ws_ps = acc_ps.tile([128, NH], f32, tag="acc")
                nc.tensor.matmul(
                    ws_ps[0:1, :], ones_sb[:, 0:1], wtexp_sb[:, :],
                    start=True, stop=True,
                )
                rcp_sb = wpool.tile([1, NH], bf16, tag="rcp")
                nc.vector.tensor_scalar_max(ws_ps[0:1, :], ws_ps[0:1, :], 1e-30)
                with nc.allow_low_precision(reason="recip rounds to bf16 on write"):
                    nc.vector.reciprocal(rcp_sb[:], ws_ps[0:1, :])
                rexp_ps = acc_ps.tile([128, NH], f32, tag="acc")
                nc.tensor.matmul(
                    rexp_ps[:, :], ones_sb[0:1, 0:128], rcp_sb[:, :],
                    start=True, stop=True,
                )
                # h*wtexp computed off the serial chain (parallel to ws->rcp->rexp)
                wh1_sb = wpool.tile([128, NH], f32, tag="wh1")
                nc.vector.tensor_tensor(
                    out=wh1_sb[:], in0=h_acc[:], in1=wtexp_sb[:], op=ALU.mult
                )
                wh_sb = wpool.tile([128, NH], bf16, tag="wh")
                nc.vector.tensor_tensor(
                    out=wh_sb[:], in0=rexp_ps[:], in1=wh1_sb[:], op=ALU.mult
                )
                if dbg and hi == 0:
                    h_f32 = wpool.tile([128, NH], f32, tag="dbgh")
                    nc.scalar.copy(out=h_f32[:], in_=h_acc[:])
                    nc.sync.dma_start(out=dbg_t["h"].ap()[:, 0:NH], in_=h_f32[:])
                    nc.sync.dma_start(out=dbg_t["wn"].ap()[:, 0:NH], in_=wn_sb[0:E, :])

                # phase 2 for this half
                for t in range(NH // 128):
                    ts = slice(t * 128, (t + 1) * 128)
                    tok0 = hi * NH + t * 128
                    o_sb = opool.tile([128, D_OUT], out_dt, tag="osb")
                    if "p2mm" in ablate:
                        nc.scalar.memzero(o_sb[:])
                    else:
                        for og in range(OC // 2):
                            po = out_ps.tile([128, 1024], f32, tag="po")
                            for j in range(2):
                                oc = og * 2 + j
                                osl = slice(oc * 512, (oc + 1) * 512)
                                nc.tensor.matmul(
                                    po[:, j * 512 : (j + 1) * 512],
                                    wh_sb[:, ts], b2_sb[:, osl],
                                    start=True, stop=True,
                                )
                            ogsl = slice(og * 1024, (og + 1) * 1024)
                            if COPY_MODE == "alt" and og % 2 == 1:
                                nc.scalar.copy(out=o_sb[:, ogsl], in_=po[:])
                            elif COPY_MODE == "any":
                                nc.any.tensor_copy(out=o_sb[:, ogsl], in_=po[:])
                            else:
                                nc.vector.tensor_copy(out=o_sb[:, ogsl], in_=po[:])
                    if "outdma" not in ablate:
                        if OUT_SPLIT == 4:
                            for q4 in range(4):
                                nc.sync.dma_start(
                                    out=out.ap()[
                                        tok0 : tok0 + 128,
                                        q4 * 1024 : (q4 + 1) * 1024,
                                    ],
                                    in_=o_sb[:, q4 * 1024 : (q4 + 1) * 1024],
                                )
                        elif OUT_SPLIT == 2:
                            nc.sync.dma_start(
                                out=out.ap()[tok0 : tok0 + 128, 0:2048],
                                in_=o_sb[:, 0:2048],
                            )
                            nc.sync.dma_start(
                                out=out.ap()[tok0 : tok0 + 128, 2048:4096],
                                in_=o_sb[:, 2048:4096],
                            )
                        else:
                            nc.sync.dma_start(
                                out=out.ap()[tok0 : tok0 + 128, :], in_=o_sb[:]
                            )

    nc.compile()
    return nc


def _make_runner(nc, n_cores=N_CORES):
    import jax
    import numpy as np
    from jax.sharding import Mesh, NamedSharding, PartitionSpec
    from jax.experimental.shard_map import shard_map
    import concourse.mybir as mybir
    from concourse.bass2jax import (
        _bass_exec_p,
        install_neuronx_cc_hook,
        partition_id_tensor,
    )

    install_neuronx_cc_hook()
    partition_name = nc.partition_id_tensor.name if nc.partition_id_tensor else None
    in_names, out_names, out_avals = [], [], []
    for alloc in nc.m.functions[0].allocations:
        if not isinstance(alloc, mybir.MemoryLocationSet):
            continue
        name = alloc.memorylocations[0].name
        if alloc.kind == "ExternalInput":
            if name != partition_name:
                in_names.append(name)
        elif alloc.kind == "ExternalOutput":
            out_names.append(name)
            out_avals.append(
                jax.core.ShapedArray(
                    tuple(alloc.tensor_shape), mybir.dt.np(alloc.dtype)
                )
            )
    n_params = len(in_names)
    n_outs = len(out_avals)
    all_in_names = in_names + out_names + ([partition_name] if partition_name else [])

    def _body(*args):
        operands = list(args)
        if partition_name is not None:
            operands.append(partition_id_tensor())
        outs = _bass_exec_p.bind(
            *operands,
            out_avals=tuple(out_avals),
            in_names=tuple(all_in_names),
            out_names=tuple(out_names),
            lowering_input_output_aliases=(),
            sim_require_finite=True,
            sim_require_nnan=True,
            nc=nc,
        )
        return tuple(outs)

    devices = jax.devices()[:n_cores]
    mesh = Mesh(np.asarray(devices), ("core",))
    sharding = NamedSharding(mesh, PartitionSpec("core"))
    in_specs = (PartitionSpec("core"),) * (n_params + n_outs)
    out_specs = (PartitionSpec("core"),) * n_outs

    fn1 = jax.jit(
        shard_map(
            _body, mesh=mesh, in_specs=in_specs, out_specs=out_specs, check_rep=False
        ),
        donate_argnums=tuple(range(n_params, n_params + n_outs)),
        keep_unused=True,
    )

    def _chain_factory(k):
        def chain(*args):
            ins = list(args[:n_params])
            z = list(args[n_params:])
            for _ in range(k):
                z = list(_body(*ins, *z))
            return tuple(z)

        return jax.jit(
            shard_map(
                chain,
                mesh=mesh,
                in_specs=in_specs,
                out_specs=out_specs,
                check_rep=False,
            ),
            donate_argnums=tuple(range(n_params, n_params + n_outs)),
            keep_unused=True,
        )

    return {
        "fn1": fn1,
        "chain_factory": _chain_factory,
        "in_names": in_names,
        "out_names": out_names,
        "out_avals": out_avals,
        "mesh": mesh,
        "sharding": sharding,
        "n_params": n_params,
    }


def _get_runner():
    if "runner" not in _CACHE:
        nc = _build_nc()
        _CACHE["nc"] = nc
        _CACHE["runner"] = _make_runner(nc)
    return _CACHE["runner"]


def _prep_inputs(inputs, router_w, router_b, thr_w, thr_b, lora_A, lora_B):
    """Host-side staging: transposes + bf16 casts + sharding. Returns dict of
    global (concatenated along axis 0) arrays."""
    import ml_dtypes

    bf16 = ml_dtypes.bfloat16
    x = np.asarray(inputs, dtype=np.float32)
    xT = np.ascontiguousarray(x.T.astype(bf16)).reshape(DC, 128, N_TOKENS)
    # A2[d, (e, r)] with e-major columns
    A2 = (
        np.ascontiguousarray(np.asarray(lora_A, np.float32).reshape(ER, D_IN).T)
        .astype(bf16)
        .reshape(DC, 128, ER)
    )
    # W9[d, 0:8]=router, [d, 8]=thr, padded to 16 cols
    W9 = np.zeros((D_IN, 40), np.float32)
    W9[:, 0:8] = np.asarray(router_w, np.float32).T
    W9[:, 32] = np.asarray(thr_w, np.float32)[0]
    W9 = W9.astype(bf16).reshape(DC, 128, 40)
    B9 = np.zeros((33, 1), np.float32)
    B9[0:8, 0] = np.asarray(router_b, np.float32)
    B9[32, 0] = np.asarray(thr_b, np.float32)[0]
    # B2[(e, r), o], pre-scaled (the extra 16 undoes wsum16 = 16*wsum)
    B2 = np.ascontiguousarray(
        np.asarray(lora_B, np.float32).transpose(0, 2, 1).reshape(ER, D_OUT)
        * (SCALING * 16.0)
    ).astype(bf16)
    G9m = np.zeros((9, ER), np.float32)
    for e in range(E):
        G9m[e, e * R : (e + 1) * R] = 1.0
    G9m[8, :] = -1.0
    G9m = G9m.astype(bf16)
    per_core = {
        "A2": A2,
        "W9": W9,
        "B9": B9,
        "B2": B2,
        "G9": G9m,
    }
    arrays = {}
    for name in ("A2", "W9", "B9", "B2", "G9"):
        a = per_core[name]
        arrays[name] = np.broadcast_to(
            a, (N_CORES,) + a.shape
        ).reshape((N_CORES * a.shape[0],) + a.shape[1:])
    # xT shards: tokens split along the last axis -> per-core [DC, 128, NL]
    xT_shards = [
        np.ascontiguousarray(xT[:, :, c * NL : (c + 1) * NL]) for c in range(N_CORES)
    ]
    arrays["xT"] = np.concatenate(xT_shards, axis=0)
    return arrays


def kernel(inputs, router_w, router_b, thr_w, thr_b, lora_A, lora_B):
    import jax

    r = _get_runner()
    arrays = _prep_inputs(
        inputs, router_w, router_b, thr_w, thr_b, lora_A, lora_B
    )
    ins = [arrays[name] for name in r["in_names"]]
    zeros = [
        np.zeros((N_CORES * a.shape[0],) + a.shape[1:], a.dtype)
        for a in r["out_avals"]
    ]
    outs = r["fn1"](*ins, *zeros)
    out = np.asarray(jax.block_until_ready(outs[0]))
    return out.reshape(N_TOKENS, D_OUT).astype(np.float32)



# revision 24
# speedup vs baseline: 1.9629x; 1.9629x over previous
"""AdaMoLE forward on 8 Trainium2 NeuronCores (Bass/Tile), data-parallel over tokens.

Reference computation (per token n):
  logits = x @ router_w.T + router_b            [N, E]
  gate   = softmax(logits)                      [N, E]
  thr    = sigmoid(x @ thr_w.T + thr_b)/E       [N, 1]
  w      = relu(gate - thr); w /= max(sum(w), eps-guard)
  h      = einsum('nd,erd->ner', x, lora_A)
  out    = einsum('ner,eor->no', h * w[:, :, None], lora_B) * SCALING

Device mapping (per core, NL=1024 tokens, two token halves pipelined).
This is a DMA-roofline kernel (~53 us of HBM traffic/core at 358 GB/s:
8 MiB x-in + 8 MiB out + ~2.3 MiB weights, all bf16). The design goals are
(a) the input DMA queue never stalls, (b) the gating serial chain is short
enough to hide inside the out-stream window, (c) the out stream starts the
moment the x stream ends and never starves.

Key structural points vs a naive mapping:
  - All weights are host-side re-laid so every DMA is fully contiguous
    per partition (A2/W9 as [128, DC, cols]); the a2 remainder and b2 are
    inserted into the serial DMA queue exactly where they are first needed.
  - Gating runs in the [e, tok] domain with a 4-hop serial chain:
    ACT(exp + tanh) -> DVE(m) -> PE(p1, 2 accumulating matmuls) ->
    DVE(relu, wh) -> PE(phase2).  The softmax denominator cancels in the
    normalization; the threshold sigmoid is computed as tanh(z/2) (same ACT
    function table as exp -- sigmoid lives in a different table set and a
    mid-chain LoadActFuncSet costs 1.3 us), with thr_w replicated across 8
    router-style columns so DVE can form m_e = (1+tanh)*ex_e lane-wise:
      p1[er] = ex[e(er)] - (1/16)*sum_e m_e = ex - MAX_T*sigmoid*S.
  - Normalization by wsum happens at the END, folded into the PSUM->SBUF
    copy: per 128-token chunk a 1-column matmul computes ws[tok,1] =
    sum_er relu(p1), DVE takes a guarded reciprocal, and the phase-2 copies
    apply it via the ACT scale port / DVE tensor_scalar -- zero extra chain
    hops, removing the ws->rcp->broadcast PE round-trips of the direct form.
  - Both halves' router/threshold logits accumulate into ONE PSUM bank
    (half A at partitions 0:40, half B at 64:104) so the PSUM budget fits:
    2 h banks + 1 lg bank + 1 rotating (p1/ws) + 2x2 po banks = 8.
  - Phase-2 A is interleaved one og-unit per xtB DMA group so the PE slack
    while waiting on xtB produces out-slices early; out-DMA slices fire per
    og the moment their copy lands.  Half-A copies go to ACT (keeping DVE
    free for gating B), half-B copies alternate ACT/DVE.
"""

import numpy as np

N_TOKENS = 8192
D_IN = 4096
D_OUT = 4096
E = 8
R = 16
ER = E * R  # 128
SCALING = 32.0 / 16.0
MAX_THRESHOLD = 1.0 / E
N_CORES = 8
NL = N_TOKENS // N_CORES  # tokens per core
DC = D_IN // 128  # 32 d-chunks
OC = D_OUT // 512  # 8 output column chunks

_CACHE = {}

# tuning knobs (read at _build_nc time)
XT_GRP = 4  # d-chunks per xt DMA
A2Q1 = 8  # d-chunks of lora_A loaded before the x stream starts
A2REST_AFTER = 1  # xtA group index after which the a2 remainder DMA is queued
U_PER_BG = 2  # phase2-A 512-col units issued per xtB DMA group
U_MID = 2  # phase2-A units issued between gating-B part 1 and p1_B
OSB_BUFS = 4
PO_BUFS = 4
OUT_BF16 = True


def _build_nc(dbg=False, repeat=1, ablate=frozenset()):
    import concourse.mybir as mybir
    import concourse.tile as tile
    from concourse import bacc

    f32 = mybir.dt.float32
    bf16 = mybir.dt.bfloat16
    AF = mybir.ActivationFunctionType
    ALU = mybir.AluOpType

    nc = bacc.Bacc("TRN2", target_bir_lowering=False, debug=False)

    xT = nc.declare_dram_parameter("xT", [DC, 128, NL], bf16, isOutput=False)
    A2 = nc.declare_dram_parameter("A2", [128, DC, ER], bf16, isOutput=False)
    W9 = nc.declare_dram_parameter("W9", [128, DC, 40], bf16, isOutput=False)
    B9 = nc.declare_dram_parameter("B9", [128, 1], f32, isOutput=False)
    GD = nc.declare_dram_parameter("GD", [8, ER], bf16, isOutput=False)
    B2 = nc.declare_dram_parameter("B2", [ER, D_OUT], bf16, isOutput=False)
    out_dt = bf16 if OUT_BF16 else f32
    out = nc.declare_dram_parameter("out", [NL, D_OUT], out_dt, isOutput=True)

    NH = NL // 2  # tokens per half
    TC = NH // 128  # 128-token chunks per half (4)
    NU = TC * 8  # 512-col units per half (32), unit u = (t=u//8, sc=u%8)
    GRP = XT_GRP
    NG = DC // GRP  # xt DMA groups per half (8)

    with tile.TileContext(nc) as tc:
        with (
            tc.tile_pool(name="const", bufs=1) as cpool,
            tc.tile_pool(name="xt", bufs=2 * NG) as xpool,
            tc.tile_pool(name="work", bufs=2) as wpool,
            tc.tile_pool(name="osb", bufs=OSB_BUFS) as opool,
            tc.tile_pool(name="hacc", bufs=2, space="PSUM") as hpool,
            tc.tile_pool(name="lg", bufs=1, space="PSUM") as lgpool,
            tc.tile_pool(name="rot", bufs=1, space="PSUM") as rotpool,
            tc.tile_pool(name="po", bufs=PO_BUFS, space="PSUM") as popool,
        ):
          for _rep in range(repeat):
            # ---- weight/constant loads (queue order = service order) ----
            a2_sb = cpool.tile([128, DC, ER], bf16)
            nc.sync.dma_start(out=a2_sb[:, 0:A2Q1, :], in_=A2.ap()[:, 0:A2Q1, :])
            w9_sb = cpool.tile([128, DC, 40], bf16)
            nc.sync.dma_start(out=w9_sb[:], in_=W9.ap())
            b9_sb = cpool.tile([128, 1], f32)
            nc.sync.dma_start(out=b9_sb[:], in_=B9.ap())
            gd_sb = cpool.tile([8, ER], bf16)
            nc.sync.dma_start(out=gd_sb[:], in_=GD.ap())
            gs_sb = cpool.tile([8, ER], bf16)
            nc.gpsimd.memset(gs_sb[:], -1.0 / 16.0)
            ones_sb = cpool.tile([128, 1], bf16)
            nc.gpsimd.memset(ones_sb[:], 1.0)
            b2_sb = cpool.tile([ER, D_OUT], bf16)

            h = {
                "A": hpool.tile([128, NH], f32, tag="hacc", name="h_A"),
                "B": hpool.tile([128, NH], f32, tag="hacc", name="h_B"),
            }
            # half A logits at partitions 0:40, half B at 64:104 (router cols
            # 0:8, replicated thr col at 32:40 so ACT reads stay 32-aligned)
            lg = lgpool.tile([128, NH], f32, tag="lg", name="lg")
            LOFF = {"A": 0, "B": 64}

            xts = {}

            def _xt_dma(half, hi, dg):
                tsl = slice(hi * NH, (hi + 1) * NH)
                t = xpool.tile([128, GRP, NH], bf16, tag="xt", name=f"xt{half}{dg}")
                nc.sync.dma_start(
                    out=t[:],
                    in_=xT.ap()[dg * GRP : (dg + 1) * GRP, :, tsl].rearrange(
                        "a b c -> b a c"
                    ),
                )
                for j in range(GRP):
                    xts[half, dg * GRP + j] = t[:, j, :]

            def _h_mm(half, dc):
                nc.tensor.matmul(
                    h[half][:, :], a2_sb[:, dc, :], xts[half, dc],
                    start=(dc == 0), stop=(dc == DC - 1),
                )

            def _lg_mm(half, dc):
                lo = LOFF[half]
                nc.tensor.matmul(
                    lg[lo : lo + 40, :], w9_sb[:, dc, :], xts[half, dc],
                    start=(dc == 0), stop=(dc == DC - 1),
                )

            # ---- x stream half A + phase-1 A inline ----
            for dg in range(NG):
                _xt_dma("A", 0, dg)
                if dg == A2REST_AFTER:
                    nc.sync.dma_start(
                        out=a2_sb[:, A2Q1:, :], in_=A2.ap()[:, A2Q1:, :]
                    )
                for j in range(GRP):
                    _h_mm("A", dg * GRP + j)
                    _lg_mm("A", dg * GRP + j)
            nc.sync.dma_start(out=b2_sb[:], in_=B2.ap())
            for dg in range(NG):
                _xt_dma("B", 1, dg)

            # ---- gating (4-hop serial chain per half) ----
            gt = {}

            def _gate_pre(half):
                """ACT exp/tanh + DVE m (issue early; waits on lg stop)."""
                lo = LOFF[half]
                ex = wpool.tile([8, NH], bf16, tag="ex")
                t8 = wpool.tile([8, NH], bf16, tag="t8")
                nc.scalar.activation(
                    ex[:], lg[lo : lo + 8, :], AF.Exp, bias=b9_sb[lo : lo + 8, 0:1]
                )
                nc.scalar.activation(
                    t8[:], lg[lo + 32 : lo + 40, :], AF.Tanh,
                    bias=b9_sb[lo + 32 : lo + 40, 0:1], scale=0.5,
                )
                m = wpool.tile([8, NH], bf16, tag="m")
                nc.vector.scalar_tensor_tensor(
                    out=m[:], in0=t8[:], scalar=1.0, in1=ex[:],
                    op0=ALU.add, op1=ALU.mult,
                )
                gt[half] = (ex, m)

            def _gate_p1(half):
                """PE p1 (x2 accumulating matmuls)."""
                ex, m = gt[half]
                p1 = rotpool.tile([128, NH], f32, tag="rot", name=f"p1_{half}")
                nc.tensor.matmul(p1[:, :], gd_sb[:], ex[:], start=True, stop=False)
                nc.tensor.matmul(p1[:, :], gs_sb[:], m[:], start=False, stop=True)
                return p1

            def _gate_relu(half, p1):
                """DVE relu (feeds the ws/rcp normalization branch only)."""
                wtexp = wpool.tile([128, NH], bf16, tag="wtexp")
                nc.vector.tensor_scalar_max(wtexp[:], p1[:], 0.0)
                return wtexp

            def _gate_wh(half, wtexp):
                """DVE wh = wtexp * h (h is the only PSUM operand)."""
                wh = wpool.tile([128, NH], bf16, tag="wh")
                nc.vector.tensor_tensor(
                    out=wh[:], in0=h[half][:], in1=wtexp[:], op=ALU.mult
                )
                return wh

            def _gate_ws(half, wtexp):
                """per-chunk ws matmuls + guarded reciprocal (off-chain)."""
                ws = rotpool.tile([128, TC], f32, tag="rot", name=f"ws_{half}")
                for t in range(TC):
                    nc.tensor.matmul(
                        ws[:, t : t + 1], wtexp[:, t * 128 : (t + 1) * 128],
                        ones_sb[:, 0:1], start=True, stop=True,
                    )
                wsg = wpool.tile([128, TC], f32, tag="wsg")
                nc.vector.tensor_scalar_max(wsg[:], ws[:], 1e-30)
                rcp = wpool.tile([128, TC], f32, tag="rcp")
                nc.vector.reciprocal(rcp[:], wsg[:])
                return rcp

            # ---- phase 2 units: one 512-col matmul + copy + out slice ----
            osb = {}

            def _unit(half, hi, wh, rcp, u):
                t, sc = u // 8, u % 8
                if sc == 0:
                    osb[half, t] = opool.tile(
                        [128, D_OUT], out_dt, tag="osb", name=f"o_{half}{t}"
                    )
                o_sb = osb[half, t]
                ts = slice(t * 128, (t + 1) * 128)
                tok0 = hi * NH + t * 128
                osl = slice(sc * 512, (sc + 1) * 512)
                po = popool.tile([128, 512], f32, tag="po")
                nc.tensor.matmul(
                    po[:], wh[:, ts], b2_sb[:, osl], start=True, stop=True
                )
                if u % 2:
                    nc.scalar.mul(o_sb[:, osl], po[:], rcp[:, t : t + 1])
                else:
                    nc.vector.tensor_scalar(
                        out=o_sb[:, osl], in0=po[:],
                        scalar1=rcp[:, t : t + 1], scalar2=None, op0=ALU.mult,
                    )
                if sc % 2 == 1:
                    # 1024-col out slices: smaller DMAs fall under the ~625ns
                    # per-DMA HWDGE sequencer cost and the stream loses density
                    dsl = slice((sc - 1) * 512, (sc + 1) * 512)
                    nc.sync.dma_start(
                        out=out.ap()[tok0 : tok0 + 128, dsl], in_=o_sb[:, dsl]
                    )

            _gate_pre("A")
            p1_A = _gate_p1("A")
            wtexp_A = _gate_relu("A", p1_A)
            wh_A = _gate_wh("A", wtexp_A)

            # ---- xtB window: lg_B only (h_B deferred) + phase-2 A units ----
            rcp_A = None
            un = 0
            for bg in range(NG):
                for j in range(GRP):
                    _lg_mm("B", bg * GRP + j)
                if bg == 0:
                    rcp_A = _gate_ws("A", wtexp_A)
                for _ in range(U_PER_BG):
                    _unit("A", 0, wh_A, rcp_A, un)
                    un += 1
            # ---- post-window: gating B chain + deferred h_B + rest of A ----
            _gate_pre("B")
            p1_B = None
            wtexp_B = None
            rcp_B = None
            for dc in range(DC):
                _h_mm("B", dc)
                if dc == 7:
                    p1_B = _gate_p1("B")
                    wtexp_B = _gate_relu("B", p1_B)
                if dc >= 8 and un < NU:
                    _unit("A", 0, wh_A, rcp_A, un)
                    un += 1
                if dc == 24:
                    rcp_B = _gate_ws("B", wtexp_B)
            wh_B = _gate_wh("B", wtexp_B)
            while un < NU:
                _unit("A", 0, wh_A, rcp_A, un)
                un += 1
            for u in range(NU):
                _unit("B", 1, wh_B, rcp_B, u)

    nc.compile()
    return nc


def _make_runner(nc, n_cores=N_CORES):
    import jax
    import numpy as np
    from jax.sharding import Mesh, NamedSharding, PartitionSpec
    from jax.experimental.shard_map import shard_map
    import concourse.mybir as mybir
    from concourse.bass2jax import (
        _bass_exec_p,
        install_neuronx_cc_hook,
        partition_id_tensor,
    )

    install_neuronx_cc_hook()
    partition_name = nc.partition_id_tensor.name if nc.partition_id_tensor else None
    in_names, out_names, out_avals = [], [], []
    for alloc in nc.m.functions[0].allocations:
        if not isinstance(alloc, mybir.MemoryLocationSet):
            continue
        name = alloc.memorylocations[0].name
        if alloc.kind == "ExternalInput":
            if name != partition_name:
                in_names.append(name)
        elif alloc.kind == "ExternalOutput":
            out_names.append(name)
            out_avals.append(
                jax.core.ShapedArray(
                    tuple(alloc.tensor_shape), mybir.dt.np(alloc.dtype)
                )
            )
    n_params = len(in_names)
    n_outs = len(out_avals)
    all_in_names = in_names + out_names + ([partition_name] if partition_name else [])

    def _body(*args):
        operands = list(args)
        if partition_name is not None:
            operands.append(partition_id_tensor())
        outs = _bass_exec_p.bind(
            *operands,
            out_avals=tuple(out_avals),
            in_names=tuple(all_in_names),
            out_names=tuple(out_names),
            lowering_input_output_aliases=(),
            sim_require_finite=True,
            sim_require_nnan=True,
            nc=nc,
        )
        return tuple(outs)

    devices = jax.devices()[:n_cores]
    mesh = Mesh(np.asarray(devices), ("core",))
    sharding = NamedSharding(mesh, PartitionSpec("core"))
    in_specs = (PartitionSpec("core"),) * (n_params + n_outs)
    out_specs = (PartitionSpec("core"),) * n_outs

    fn1 = jax.jit(
        shard_map(
            _body, mesh=mesh, in_specs=in_specs, out_specs=out_specs, check_rep=False
        ),
        donate_argnums=tuple(range(n_params, n_params + n_outs)),
        keep_unused=True,
    )

    def _chain_factory(k):
        def chain(*args):
            ins = list(args[:n_params])
            z = list(args[n_params:])
            for _ in range(k):
                z = list(_body(*ins, *z))
            return tuple(z)

        return jax.jit(
            shard_map(
                chain,
                mesh=mesh,
                in_specs=in_specs,
                out_specs=out_specs,
                check_rep=False,
            ),
            donate_argnums=tuple(range(n_params, n_params + n_outs)),
            keep_unused=True,
        )

    return {
        "fn1": fn1,
        "chain_factory": _chain_factory,
        "in_names": in_names,
        "out_names": out_names,
        "out_avals": out_avals,
        "mesh": mesh,
        "sharding": sharding,
        "n_params": n_params,
    }


def _get_runner():
    if "runner" not in _CACHE:
        nc = _build_nc()
        _CACHE["nc"] = nc
        _CACHE["runner"] = _make_runner(nc)
    return _CACHE["runner"]


def _prep_inputs(inputs, router_w, router_b, thr_w, thr_b, lora_A, lora_B):
    """Host-side staging: transposes + bf16 casts + sharding. Returns dict of
    global (concatenated along axis 0) arrays."""
    import ml_dtypes

    bf16 = ml_dtypes.bfloat16
    x = np.asarray(inputs, dtype=np.float32)
    xT = np.ascontiguousarray(x.T.astype(bf16)).reshape(DC, 128, N_TOKENS)
    # A2[p, dc, (e, r)]: contiguous per-partition weight lines
    A2 = np.ascontiguousarray(
        np.asarray(lora_A, np.float32).reshape(ER, DC, 128).transpose(2, 1, 0)
    ).astype(bf16)
    # W9[p, dc, 0:8]=router, [p, dc, 32:40]=thr replicated
    W9 = np.zeros((128, DC, 40), np.float32)
    W9[:, :, 0:8] = (
        np.asarray(router_w, np.float32).T.reshape(DC, 128, E).transpose(1, 0, 2)
    )
    W9[:, :, 32:40] = np.asarray(thr_w, np.float32)[0].reshape(DC, 128).T[:, :, None]
    W9 = W9.astype(bf16)
    # B9 rows 0:8 / 64:72 = router_b; rows 32:40 / 96:104 = thr_b/2 (tanh form)
    B9 = np.zeros((128, 1), np.float32)
    rb = np.asarray(router_b, np.float32)
    tb = float(np.asarray(thr_b, np.float32)[0]) * 0.5
    B9[0:8, 0] = rb
    B9[64:72, 0] = rb
    B9[32:40, 0] = tb
    B9[96:104, 0] = tb
    # GD[e, (e', r)] = delta_{e, e'} (expert expansion)
    GDm = np.zeros((8, ER), np.float32)
    for e in range(E):
        GDm[e, e * R : (e + 1) * R] = 1.0
    GDm = GDm.astype(bf16)
    # B2[(e, r), o], pre-scaled (the extra 16 undoes ws = 16*S*wsum)
    B2 = np.ascontiguousarray(
        np.asarray(lora_B, np.float32).transpose(0, 2, 1).reshape(ER, D_OUT)
        * (SCALING * 16.0)
    ).astype(bf16)
    per_core = {
        "A2": A2,
        "W9": W9,
        "B9": B9,
        "B2": B2,
        "GD": GDm,
    }
    arrays = {}
    for name in ("A2", "W9", "B9", "B2", "GD"):
        a = per_core[name]
        arrays[name] = np.broadcast_to(
            a, (N_CORES,) + a.shape
        ).reshape((N_CORES * a.shape[0],) + a.shape[1:])
    # xT shards: tokens split along the last axis -> per-core [DC, 128, NL]
    xT_shards = [
        np.ascontiguousarray(xT[:, :, c * NL : (c + 1) * NL]) for c in range(N_CORES)
    ]
    arrays["xT"] = np.concatenate(xT_shards, axis=0)
    return arrays


def kernel(inputs, router_w, router_b, thr_w, thr_b, lora_A, lora_B):
    import jax

    r = _get_runner()
    arrays = _prep_inputs(
        inputs, router_w, router_b, thr_w, thr_b, lora_A, lora_B
    )
    ins = [arrays[name] for name in r["in_names"]]
    zeros = [
        np.zeros((N_CORES * a.shape[0],) + a.shape[1:], a.dtype)
        for a in r["out_avals"]
    ]
    outs = r["fn1"](*ins, *zeros)
    out = np.asarray(jax.block_until_ready(outs[0]))
    return out.reshape(N_TOKENS, D_OUT).astype(np.float32)


# revision 28
# speedup vs baseline: 6.7836x; 3.4559x over previous
"""AdaMoLE forward on 8 Trainium2 NeuronCores (Bass/Tile), data-parallel over tokens.

Reference computation (per token n):
  logits = x @ router_w.T + router_b            [N, E]
  gate   = softmax(logits)                      [N, E]
  thr    = sigmoid(x @ thr_w.T + thr_b)/E       [N, 1]
  w      = relu(gate - thr); w /= max(sum(w), eps-guard)
  h      = einsum('nd,erd->ner', x, lora_A)
  out    = einsum('ner,eor->no', h * w[:, :, None], lora_B) * SCALING

Device mapping (per core, NL=1024 tokens, two token halves pipelined).
This is a DMA-roofline kernel (~53 us of HBM traffic/core at 358 GB/s:
8 MiB x-in + 8 MiB out + ~2.3 MiB weights, all bf16). The design goals are
(a) the input DMA queue never stalls, (b) the gating serial chain is short
enough to hide inside the out-stream window, (c) the out stream starts the
moment the x stream ends and never starves.

Key structural points vs a naive mapping:
  - All weights are host-side re-laid so every DMA is fully contiguous
    per partition (A2/W9 as [128, DC, cols]); the a2 remainder and b2 are
    inserted into the serial DMA queue exactly where they are first needed.
  - Gating runs in the [e, tok] domain with a 4-hop serial chain:
    ACT(exp + tanh) -> DVE(m) -> PE(p1, 2 accumulating matmuls) ->
    DVE(relu, wh) -> PE(phase2).  The softmax denominator cancels in the
    normalization; the threshold sigmoid is computed as tanh(z/2) (same ACT
    function table as exp -- sigmoid lives in a different table set and a
    mid-chain LoadActFuncSet costs 1.3 us), with thr_w replicated across 8
    router-style columns so DVE can form m_e = (1+tanh)*ex_e lane-wise:
      p1[er] = ex[e(er)] - (1/16)*sum_e m_e = ex - MAX_T*sigmoid*S.
  - Normalization by wsum happens at the END, folded into the PSUM->SBUF
    copy: per 128-token chunk a 1-column matmul computes ws[tok,1] =
    sum_er relu(p1), DVE takes a guarded reciprocal, and the phase-2 copies
    apply it via the ACT scale port / DVE tensor_scalar -- zero extra chain
    hops, removing the ws->rcp->broadcast PE round-trips of the direct form.
  - Both halves' router/threshold logits accumulate into ONE PSUM bank
    (half A at partitions 0:40, half B at 64:104) so the PSUM budget fits:
    2 h banks + 1 lg bank + 1 rotating (p1/ws) + 2x2 po banks = 8.
  - Phase-2 A is interleaved one og-unit per xtB DMA group so the PE slack
    while waiting on xtB produces out-slices early; out-DMA slices fire per
    og the moment their copy lands.  Half-A copies go to ACT (keeping DVE
    free for gating B), half-B copies alternate ACT/DVE.
"""

import numpy as np

N_TOKENS = 8192
D_IN = 4096
D_OUT = 4096
E = 8
R = 16
ER = E * R  # 128
SCALING = 32.0 / 16.0
MAX_THRESHOLD = 1.0 / E
N_CORES = 8
NL = N_TOKENS // N_CORES  # tokens per core
DC = D_IN // 128  # 32 d-chunks
OC = D_OUT // 512  # 8 output column chunks

_CACHE = {}

# tuning knobs (read at _build_nc time)
XT_GRP = 4  # d-chunks per xt DMA
A2Q1 = 8  # d-chunks of lora_A loaded before the x stream starts
A2REST_AFTER = 1  # xtA group index after which the a2 remainder DMA is queued
U_PER_BG = 2  # phase2-A 512-col units issued per xtB DMA group
U_MID = 2  # phase2-A units issued between gating-B part 1 and p1_B
OSB_BUFS = 4
PO_BUFS = 4
OUT_BF16 = True


def _build_nc(dbg=False, repeat=1, ablate=frozenset()):
    import concourse.mybir as mybir
    import concourse.tile as tile
    from concourse import bacc

    f32 = mybir.dt.float32
    bf16 = mybir.dt.bfloat16
    AF = mybir.ActivationFunctionType
    ALU = mybir.AluOpType

    nc = bacc.Bacc("TRN2", target_bir_lowering=False, debug=False)

    xT = nc.declare_dram_parameter("xT", [DC, 128, NL], bf16, isOutput=False)
    A2 = nc.declare_dram_parameter("A2", [128, DC, ER], bf16, isOutput=False)
    W9 = nc.declare_dram_parameter("W9", [128, DC, 40], bf16, isOutput=False)
    B9 = nc.declare_dram_parameter("B9", [128, 1], f32, isOutput=False)
    GD = nc.declare_dram_parameter("GD", [8, ER], bf16, isOutput=False)
    B2 = nc.declare_dram_parameter("B2", [ER, D_OUT], bf16, isOutput=False)
    out_dt = bf16 if OUT_BF16 else f32
    out = nc.declare_dram_parameter("out", [NL, D_OUT], out_dt, isOutput=True)

    NH = NL // 2  # tokens per half
    TC = NH // 128  # 128-token chunks per half (4)
    NU = TC * 8  # 512-col units per half (32), unit u = (t=u//8, sc=u%8)
    GRP = XT_GRP
    NG = DC // GRP  # xt DMA groups per half (8)

    with tile.TileContext(nc) as tc:
        with (
            tc.tile_pool(name="const", bufs=1) as cpool,
            tc.tile_pool(name="xt", bufs=2 * NG) as xpool,
            tc.tile_pool(name="work", bufs=2) as wpool,
            tc.tile_pool(name="osb", bufs=OSB_BUFS) as opool,
            tc.tile_pool(name="hacc", bufs=2, space="PSUM") as hpool,
            tc.tile_pool(name="lg", bufs=1, space="PSUM") as lgpool,
            tc.tile_pool(name="rot", bufs=1, space="PSUM") as rotpool,
            tc.tile_pool(name="po", bufs=PO_BUFS, space="PSUM") as popool,
        ):
          for _rep in range(repeat):
            # ---- weight/constant loads (queue order = service order) ----
            a2_sb = cpool.tile([128, DC, ER], bf16)
            nc.sync.dma_start(out=a2_sb[:, 0:A2Q1, :], in_=A2.ap()[:, 0:A2Q1, :])
            w9_sb = cpool.tile([128, DC, 40], bf16)
            nc.sync.dma_start(out=w9_sb[:], in_=W9.ap())
            b9_sb = cpool.tile([128, 1], f32)
            nc.sync.dma_start(out=b9_sb[:], in_=B9.ap())
            gd_sb = cpool.tile([8, ER], bf16)
            nc.sync.dma_start(out=gd_sb[:], in_=GD.ap())
            gs_sb = cpool.tile([8, ER], bf16)
            nc.gpsimd.memset(gs_sb[:], -1.0 / 16.0)
            ones_sb = cpool.tile([128, 1], bf16)
            nc.gpsimd.memset(ones_sb[:], 1.0)
            b2_sb = cpool.tile([ER, D_OUT], bf16)

            h = {
                "A": hpool.tile([128, NH], f32, tag="hacc", name="h_A"),
                "B": hpool.tile([128, NH], f32, tag="hacc", name="h_B"),
            }
            # half A logits at partitions 0:40, half B at 64:104 (router cols
            # 0:8, replicated thr col at 32:40 so ACT reads stay 32-aligned)
            lg = lgpool.tile([128, NH], f32, tag="lg", name="lg")
            LOFF = {"A": 0, "B": 64}

            xts = {}

            def _xt_dma(half, hi, dg):
                tsl = slice(hi * NH, (hi + 1) * NH)
                t = xpool.tile([128, GRP, NH], bf16, tag="xt", name=f"xt{half}{dg}")
                nc.sync.dma_start(
                    out=t[:],
                    in_=xT.ap()[dg * GRP : (dg + 1) * GRP, :, tsl].rearrange(
                        "a b c -> b a c"
                    ),
                )
                for j in range(GRP):
                    xts[half, dg * GRP + j] = t[:, j, :]

            def _h_mm(half, dc, csl=slice(0, NH)):
                nc.tensor.matmul(
                    h[half][:, csl], a2_sb[:, dc, :], xts[half, dc][:, csl],
                    start=(dc == 0), stop=(dc == DC - 1),
                )

            def _lg_mm(half, dc):
                lo = LOFF[half]
                nc.tensor.matmul(
                    lg[lo : lo + 40, :], w9_sb[:, dc, :], xts[half, dc],
                    start=(dc == 0), stop=(dc == DC - 1),
                )

            # ---- x stream half A + phase-1 A inline ----
            for dg in range(NG):
                _xt_dma("A", 0, dg)
                if dg == A2REST_AFTER:
                    nc.sync.dma_start(
                        out=a2_sb[:, A2Q1:, :], in_=A2.ap()[:, A2Q1:, :]
                    )
                for j in range(GRP):
                    _h_mm("A", dg * GRP + j)
                    _lg_mm("A", dg * GRP + j)
            nc.sync.dma_start(out=b2_sb[:], in_=B2.ap())
            for dg in range(NG):
                _xt_dma("B", 1, dg)

            # ---- gating (4-hop serial chain per half) ----
            gt = {}

            def _gate_pre(half):
                """ACT exp/tanh + DVE m (issue early; waits on lg stop)."""
                lo = LOFF[half]
                ex = wpool.tile([8, NH], bf16, tag="ex")
                t8 = wpool.tile([8, NH], bf16, tag="t8")
                nc.scalar.activation(
                    ex[:], lg[lo : lo + 8, :], AF.Exp, bias=b9_sb[lo : lo + 8, 0:1]
                )
                nc.scalar.activation(
                    t8[:], lg[lo + 32 : lo + 40, :], AF.Tanh,
                    bias=b9_sb[lo + 32 : lo + 40, 0:1], scale=0.5,
                )
                m = wpool.tile([8, NH], bf16, tag="m")
                nc.vector.scalar_tensor_tensor(
                    out=m[:], in0=t8[:], scalar=1.0, in1=ex[:],
                    op0=ALU.add, op1=ALU.mult,
                )
                gt[half] = (ex, m)

            def _gate_p1(half):
                """PE p1 (x2 accumulating matmuls)."""
                ex, m = gt[half]
                p1 = rotpool.tile([128, NH], f32, tag="rot", name=f"p1_{half}")
                nc.tensor.matmul(p1[:, :], gd_sb[:], ex[:], start=True, stop=False)
                nc.tensor.matmul(p1[:, :], gs_sb[:], m[:], start=False, stop=True)
                return p1

            def _gate_relu(half, p1):
                """DVE relu (feeds the ws/rcp normalization branch only)."""
                wtexp = wpool.tile([128, NH], bf16, tag="wtexp")
                nc.vector.tensor_scalar_max(wtexp[:], p1[:], 0.0)
                return wtexp

            def _gate_wh(half, wtexp, csl=slice(0, NH), wh=None):
                """DVE wh = wtexp * h (h is the only PSUM operand)."""
                if wh is None:
                    wh = wpool.tile([128, NH], bf16, tag="wh")
                nc.vector.tensor_tensor(
                    out=wh[:, csl], in0=h[half][:, csl], in1=wtexp[:, csl],
                    op=ALU.mult,
                )
                return wh

            def _gate_ws(half, wtexp):
                """per-chunk ws matmuls + guarded reciprocal (off-chain)."""
                ws = rotpool.tile([128, TC], f32, tag="rot", name=f"ws_{half}")
                for t in range(TC):
                    nc.tensor.matmul(
                        ws[:, t : t + 1], wtexp[:, t * 128 : (t + 1) * 128],
                        ones_sb[:, 0:1], start=True, stop=True,
                    )
                wsg = wpool.tile([128, TC], f32, tag="wsg")
                nc.vector.tensor_scalar_max(wsg[:], ws[:], 1e-30)
                rcp = wpool.tile([128, TC], f32, tag="rcp")
                nc.vector.reciprocal(rcp[:], wsg[:])
                return rcp

            # ---- phase 2 units: one 512-col matmul + copy + out slice ----
            osb = {}

            def _unit(half, hi, wh, rcp, u):
                t, sc = u // 8, u % 8
                if sc == 0:
                    osb[half, t] = opool.tile(
                        [128, D_OUT], out_dt, tag="osb", name=f"o_{half}{t}"
                    )
                o_sb = osb[half, t]
                ts = slice(t * 128, (t + 1) * 128)
                tok0 = hi * NH + t * 128
                osl = slice(sc * 512, (sc + 1) * 512)
                po = popool.tile([128, 512], f32, tag="po")
                nc.tensor.matmul(
                    po[:], wh[:, ts], b2_sb[:, osl], start=True, stop=True
                )
                if u % 2:
                    nc.scalar.mul(o_sb[:, osl], po[:], rcp[:, t : t + 1])
                else:
                    nc.vector.tensor_scalar(
                        out=o_sb[:, osl], in0=po[:],
                        scalar1=rcp[:, t : t + 1], scalar2=None, op0=ALU.mult,
                    )
                if sc % 2 == 1:
                    # 1024-col out slices: smaller DMAs fall under the ~625ns
                    # per-DMA HWDGE sequencer cost and the stream loses density
                    dsl = slice((sc - 1) * 512, (sc + 1) * 512)
                    nc.sync.dma_start(
                        out=out.ap()[tok0 : tok0 + 128, dsl], in_=o_sb[:, dsl]
                    )

            _gate_pre("A")
            p1_A = _gate_p1("A")
            wtexp_A = _gate_relu("A", p1_A)
            wh_A = _gate_wh("A", wtexp_A)

            # ---- xtB window: lg_B only (h_B deferred) + phase-2 A units ----
            rcp_A = None
            un = 0
            for bg in range(NG):
                for j in range(GRP):
                    _lg_mm("B", bg * GRP + j)
                if bg == 0:
                    rcp_A = _gate_ws("A", wtexp_A)
                for _ in range(U_PER_BG):
                    _unit("A", 0, wh_A, rcp_A, un)
                    un += 1
            # ---- post-window: gating B chain + deferred h_B + rest of A ----
            # h_B accumulates in two 256-token column groups so wh for the
            # first two token chunks lands early and uB production overlaps
            # the second group's accumulation
            _gate_pre("B")
            CH = NH // 2
            c1, c2 = slice(0, CH), slice(CH, NH)
            p1_B = None
            wtexp_B = None
            rcp_B = None
            for dc in range(DC):
                _h_mm("B", dc, c1)
                if dc == 7:
                    p1_B = _gate_p1("B")
                    wtexp_B = _gate_relu("B", p1_B)
                if dc >= 8 and un < NU:
                    _unit("A", 0, wh_A, rcp_A, un)
                    un += 1
                if dc == 12:
                    rcp_B = _gate_ws("B", wtexp_B)
            wh_B = _gate_wh("B", wtexp_B, c1)
            while un < NU:
                _unit("A", 0, wh_A, rcp_A, un)
                un += 1
            ub = 0
            for dc in range(DC):
                if ub < NU // 2:
                    _unit("B", 1, wh_B, rcp_B, ub)
                    ub += 1
                _h_mm("B", dc, c2)
            _gate_wh("B", wtexp_B, c2, wh=wh_B)
            while ub < NU:
                _unit("B", 1, wh_B, rcp_B, ub)
                ub += 1

    nc.compile()
    return nc


def _make_runner(nc, n_cores=N_CORES):
    import jax
    import numpy as np
    from jax.sharding import Mesh, NamedSharding, PartitionSpec
    from jax.experimental.shard_map import shard_map
    import concourse.mybir as mybir
    from concourse.bass2jax import (
        _bass_exec_p,
        install_neuronx_cc_hook,
        partition_id_tensor,
    )

    install_neuronx_cc_hook()
    partition_name = nc.partition_id_tensor.name if nc.partition_id_tensor else None
    in_names, out_names, out_avals = [], [], []
    for alloc in nc.m.functions[0].allocations:
        if not isinstance(alloc, mybir.MemoryLocationSet):
            continue
        name = alloc.memorylocations[0].name
        if alloc.kind == "ExternalInput":
            if name != partition_name:
                in_names.append(name)
        elif alloc.kind == "ExternalOutput":
            out_names.append(name)
            out_avals.append(
                jax.core.ShapedArray(
                    tuple(alloc.tensor_shape), mybir.dt.np(alloc.dtype)
                )
            )
    n_params = len(in_names)
    n_outs = len(out_avals)
    all_in_names = in_names + out_names + ([partition_name] if partition_name else [])

    def _body(*args):
        operands = list(args)
        if partition_name is not None:
            operands.append(partition_id_tensor())
        outs = _bass_exec_p.bind(
            *operands,
            out_avals=tuple(out_avals),
            in_names=tuple(all_in_names),
            out_names=tuple(out_names),
            lowering_input_output_aliases=(),
            sim_require_finite=True,
            sim_require_nnan=True,
            nc=nc,
        )
        return tuple(outs)

    devices = jax.devices()[:n_cores]
    mesh = Mesh(np.asarray(devices), ("core",))
    sharding = NamedSharding(mesh, PartitionSpec("core"))
    in_specs = (PartitionSpec("core"),) * (n_params + n_outs)
    out_specs = (PartitionSpec("core"),) * n_outs

    fn1 = jax.jit(
        shard_map(
            _body, mesh=mesh, in_specs=in_specs, out_specs=out_specs, check_rep=False
        ),
        donate_argnums=tuple(range(n_params, n_params + n_outs)),
        keep_unused=True,
    )

    def _chain_factory(k):
        def chain(*args):
            ins = list(args[:n_params])
            z = list(args[n_params:])
            for _ in range(k):
                z = list(_body(*ins, *z))
            return tuple(z)

        return jax.jit(
            shard_map(
                chain,
                mesh=mesh,
                in_specs=in_specs,
                out_specs=out_specs,
                check_rep=False,
            ),
            donate_argnums=tuple(range(n_params, n_params + n_outs)),
            keep_unused=True,
        )

    return {
        "fn1": fn1,
        "chain_factory": _chain_factory,
        "in_names": in_names,
        "out_names": out_names,
        "out_avals": out_avals,
        "mesh": mesh,
        "sharding": sharding,
        "n_params": n_params,
    }


def _get_runner():
    if "runner" not in _CACHE:
        nc = _build_nc()
        _CACHE["nc"] = nc
        _CACHE["runner"] = _make_runner(nc)
    return _CACHE["runner"]


def _prep_inputs(inputs, router_w, router_b, thr_w, thr_b, lora_A, lora_B):
    """Host-side staging: transposes + bf16 casts + sharding. Returns dict of
    global (concatenated along axis 0) arrays."""
    import ml_dtypes

    bf16 = ml_dtypes.bfloat16
    x = np.asarray(inputs, dtype=np.float32)
    xT = np.ascontiguousarray(x.T.astype(bf16)).reshape(DC, 128, N_TOKENS)
    # A2[p, dc, (e, r)]: contiguous per-partition weight lines
    A2 = np.ascontiguousarray(
        np.asarray(lora_A, np.float32).reshape(ER, DC, 128).transpose(2, 1, 0)
    ).astype(bf16)
    # W9[p, dc, 0:8]=router, [p, dc, 32:40]=thr replicated
    W9 = np.zeros((128, DC, 40), np.float32)
    W9[:, :, 0:8] = (
        np.asarray(router_w, np.float32).T.reshape(DC, 128, E).transpose(1, 0, 2)
    )
    W9[:, :, 32:40] = np.asarray(thr_w, np.float32)[0].reshape(DC, 128).T[:, :, None]
    W9 = W9.astype(bf16)
    # B9 rows 0:8 / 64:72 = router_b; rows 32:40 / 96:104 = thr_b/2 (tanh form)
    B9 = np.zeros((128, 1), np.float32)
    rb = np.asarray(router_b, np.float32)
    tb = float(np.asarray(thr_b, np.float32)[0]) * 0.5
    B9[0:8, 0] = rb
    B9[64:72, 0] = rb
    B9[32:40, 0] = tb
    B9[96:104, 0] = tb
    # GD[e, (e', r)] = delta_{e, e'} (expert expansion)
    GDm = np.zeros((8, ER), np.float32)
    for e in range(E):
        GDm[e, e * R : (e + 1) * R] = 1.0
    GDm = GDm.astype(bf16)
    # B2[(e, r), o], pre-scaled (the extra 16 undoes ws = 16*S*wsum)
    B2 = np.ascontiguousarray(
        np.asarray(lora_B, np.float32).transpose(0, 2, 1).reshape(ER, D_OUT)
        * (SCALING * 16.0)
    ).astype(bf16)
    per_core = {
        "A2": A2,
        "W9": W9,
        "B9": B9,
        "B2": B2,
        "GD": GDm,
    }
    arrays = {}
    for name in ("A2", "W9", "B9", "B2", "GD"):
        a = per_core[name]
        arrays[name] = np.broadcast_to(
            a, (N_CORES,) + a.shape
        ).reshape((N_CORES * a.shape[0],) + a.shape[1:])
    # xT shards: tokens split along the last axis -> per-core [DC, 128, NL]
    xT_shards = [
        np.ascontiguousarray(xT[:, :, c * NL : (c + 1) * NL]) for c in range(N_CORES)
    ]
    arrays["xT"] = np.concatenate(xT_shards, axis=0)
    return arrays


def kernel(inputs, router_w, router_b, thr_w, thr_b, lora_A, lora_B):
    import jax

    r = _get_runner()
    arrays = _prep_inputs(
        inputs, router_w, router_b, thr_w, thr_b, lora_A, lora_B
    )
    ins = [arrays[name] for name in r["in_names"]]
    zeros = [
        np.zeros((N_CORES * a.shape[0],) + a.shape[1:], a.dtype)
        for a in r["out_avals"]
    ]
    outs = r["fn1"](*ins, *zeros)
    out = np.asarray(jax.block_until_ready(outs[0]))
    return out.reshape(N_TOKENS, D_OUT).astype(np.float32)
